# revision 1
# baseline (speedup 1.0000x reference)
"""Grouped-experts SwiGLU MoE kernel for Trainium2 (8 NeuronCores).

Problem: T=8192 tokens (pre-sorted into contiguous per-expert blocks of
sizes num_tokens_per_expert), D=1024, H=2816, E=8 experts.
out[t] = (silu(x@w1^T) * (x@w3^T)) @ w2^T  with the owning expert's weights;
tokens past sum(counts) produce zeros.

Sharding: 8-way tensor-parallel split of the hidden dim H (padded
2816 -> 3072 = 24 tiles of 128; each core owns 3 h-tiles of every expert).
Every core processes ALL valid tokens of ALL experts for its h-slice and
emits partial outputs (contraction over h is split); the host sums the 8
partials.  This makes every core's instruction stream identical (true SPMD)
while doing only ceil(count_e/128)-tile work per expert -- perfectly
load-balanced regardless of how unbalanced the expert counts are.

GEMMs run in bf16 (PE 1 cycle/row vs 4 for fp32) with fp32 PSUM accumulation.
"""

import sys

sys.path.insert(0, "/opt/trn_rl_repo")

import numpy as np
import ml_dtypes

T, D, E = 8192, 1024, 8
H = 2816
CAP = T // E
NCORES = 8
HT = 3  # h-tiles of 128 per core (24 total, 22 real + 2 zero pads)
HSLICE = HT * 128  # 384
BF16 = ml_dtypes.bfloat16

_COMPILE_CACHE = {}
LAST_RESULTS = None  # BassKernelResults of the most recent device run


def _derive_cfg(counts):
    """Static structure derived from the per-expert token counts."""
    counts = [int(c) for c in counts]
    tiles = [-(-c // 128) for c in counts]  # ceil(c/128), 0..8
    offs = [0]
    for t in tiles:
        offs.append(offs[-1] + t * 128)
    total_cols = offs[-1]
    # chunks of <=512 packed-x columns, each within a single expert
    chunks = []  # (expert, col0, width)
    for e in range(E):
        cols = tiles[e] * 128
        c0 = 0
        while c0 < cols:
            w = min(512, cols - c0)
            chunks.append((e, offs[e] + c0, w))
            c0 += w
    return {
        "counts": counts,
        "tiles": tiles,
        "offs": offs[:E],
        "total_cols": total_cols,
        "chunks": chunks,
    }


def _build_program(cfg):
    import concourse.bass as bass
    import concourse.bacc as bacc
    import concourse.mybir as mybir
    import concourse.tile as tile

    dt = mybir.dt
    COLS = cfg["total_cols"]
    tiles = cfg["tiles"]
    offs = cfg["offs"]

    nc = bacc.Bacc("TRN2", target_bir_lowering=False, debug=False,
                   num_devices=NCORES)

    xts = nc.dram_tensor("xts", [D, COLS], dt.bfloat16, kind="ExternalInput").ap()
    w1s = nc.dram_tensor("w1s", [E, D, HSLICE], dt.bfloat16, kind="ExternalInput").ap()
    w3s = nc.dram_tensor("w3s", [E, D, HSLICE], dt.bfloat16, kind="ExternalInput").ap()
    w2s = nc.dram_tensor("w2s", [E, HSLICE, D], dt.bfloat16, kind="ExternalInput").ap()
    outp = nc.dram_tensor("outp", [COLS, D], dt.bfloat16, kind="ExternalOutput").ap()

    with tile.TileContext(nc) as tc:
        with (
            tc.tile_pool(name="xpool", bufs=1) as xpool,
            tc.tile_pool(name="wpool", bufs=2) as wpool,
            tc.tile_pool(name="h2pool", bufs=2) as h2pool,
            tc.tile_pool(name="sgpool", bufs=3) as sgpool,
            tc.tile_pool(name="obpool", bufs=3) as obpool,
            tc.tile_pool(name="psgu", bufs=2, space="PSUM") as psgu,
            tc.tile_pool(name="pso", bufs=2, space="PSUM") as pso,
        ):
            # resident packed-x tiles: d-tile -> (128, COLS)
            xt = []
            for do in range(8):
                t = xpool.tile([128, COLS], dt.bfloat16, tag=f"xt{do}")
                nc.sync.dma_start(t[:], xts[do * 128:(do + 1) * 128, :])
                xt.append(t)

            for e in range(E):
                if tiles[e] == 0:
                    continue
                ecols = tiles[e] * 128
                w1t = wpool.tile([128, 8, HSLICE], dt.bfloat16, tag="w1t")
                nc.sync.dma_start(
                    w1t[:], w1s[e].rearrange("(do p) h -> p do h", p=128))
                w3t = wpool.tile([128, 8, HSLICE], dt.bfloat16, tag="w3t")
                nc.sync.dma_start(
                    w3t[:], w3s[e].rearrange("(do p) h -> p do h", p=128))
                w2t = wpool.tile([128, HT, D], dt.bfloat16, tag="w2t")
                nc.sync.dma_start(
                    w2t[:], w2s[e].rearrange("(ko p) d -> p ko d", p=128))

                h2 = h2pool.tile([128, HT, 1024], dt.bfloat16, tag="h2")
                for h in range(HT):
                    for (ce, col0, w) in cfg["chunks"]:
                        if ce != e:
                            continue
                        rel0 = col0 - offs[e]
                        pg = psgu.tile([128, 512], dt.float32, tag="pg")
                        pu = psgu.tile([128, 512], dt.float32, tag="pu")
                        for d in range(8):
                            nc.tensor.matmul(
                                pg[:, :w],
                                w1t[:, d, h * 128:(h + 1) * 128],
                                xt[d][:, col0:col0 + w],
                                start=(d == 0), stop=(d == 7))
                        for d in range(8):
                            nc.tensor.matmul(
                                pu[:, :w],
                                w3t[:, d, h * 128:(h + 1) * 128],
                                xt[d][:, col0:col0 + w],
                                start=(d == 0), stop=(d == 7))
                        sg = sgpool.tile([128, 512], dt.float32, tag="sg")
                        nc.scalar.activation(
                            sg[:, :w], pg[:, :w],
                            mybir.ActivationFunctionType.Silu)
                        nc.vector.tensor_mul(
                            out=h2[:, h, rel0:rel0 + w],
                            in0=sg[:, :w], in1=pu[:, :w])

                for tt in range(tiles[e]):
                    row0 = offs[e] + tt * 128
                    ob = obpool.tile([128, D], dt.bfloat16, tag="ob")
                    for dh in range(2):
                        po = pso.tile([128, 512], dt.float32, tag="po")
                        for k in range(HT):
                            nc.tensor.matmul(
                                po[:],
                                h2[:, k, tt * 128:(tt + 1) * 128],
                                w2t[:, k, dh * 512:(dh + 1) * 512],
                                start=(k == 0), stop=(k == HT - 1))
                        nc.vector.tensor_copy(
                            out=ob[:, dh * 512:(dh + 1) * 512], in_=po[:])
                    nc.sync.dma_start(outp[row0:row0 + 128, :], ob[:])

    nc.compile()
    return nc


def _get_program(cfg):
    key = tuple(cfg["tiles"])
    if key not in _COMPILE_CACHE:
        _COMPILE_CACHE[key] = _build_program(cfg)
    return _COMPILE_CACHE[key]


def _pack_inputs(x, counts, w1, w2, w3, cfg):
    """Build per-core input maps (host-side routing + layout)."""
    tiles, offs, COLS = cfg["tiles"], cfg["offs"], cfg["total_cols"]

    # packed x: all valid tokens, per-expert blocks padded to 128 rows
    xpack = np.zeros((COLS, D), np.float32)
    starts = np.concatenate([[0], np.cumsum(counts)]).astype(np.int64)
    for e in range(E):
        c = int(counts[e])
        if c:
            xpack[offs[e]:offs[e] + c] = x[starts[e]:starts[e] + c]
    xts = np.ascontiguousarray(xpack.T.astype(BF16))  # (D, COLS)

    # weights: transpose so the contraction dim leads, pad H to 3072,
    # slice per core
    w1b = w1.astype(BF16)
    w3b = w3.astype(BF16)
    w2b = w2.astype(BF16)
    # (E, D, Hpad)
    w1T = np.zeros((E, D, NCORES * HSLICE), BF16)
    w1T[:, :, :H] = np.transpose(w1b, (0, 2, 1))
    w3T = np.zeros((E, D, NCORES * HSLICE), BF16)
    w3T[:, :, :H] = np.transpose(w3b, (0, 2, 1))
    # (E, Hpad, D)
    w2T = np.zeros((E, NCORES * HSLICE, D), BF16)
    w2T[:, :H, :] = np.transpose(w2b, (0, 2, 1))

    in_maps = []
    for c in range(NCORES):
        hs = slice(c * HSLICE, (c + 1) * HSLICE)
        in_maps.append({
            "xts": xts,
            "w1s": np.ascontiguousarray(w1T[:, :, hs]),
            "w3s": np.ascontiguousarray(w3T[:, :, hs]),
            "w2s": np.ascontiguousarray(w2T[:, hs, :]),
        })
    return in_maps, starts


def _unpack_output(results, counts, cfg, starts):
    offs = cfg["offs"]
    acc = np.zeros((cfg["total_cols"], D), np.float32)
    for r in results:
        acc += r["outp"].astype(np.float32)
    out = np.zeros((T, D), np.float32)
    for e in range(E):
        c = int(counts[e])
        if c:
            out[starts[e]:starts[e] + c] = acc[offs[e]:offs[e] + c]
    return out


def kernel(x, num_tokens_per_expert, w1, w2, w3):
    global LAST_RESULTS
    counts = np.asarray(num_tokens_per_expert).astype(np.int64)
    cfg = _derive_cfg(counts)
    if cfg["total_cols"] == 0:
        return np.zeros((T, D), np.float32)

    nc = _get_program(cfg)
    in_maps, starts = _pack_inputs(
        np.asarray(x, np.float32), counts,
        np.asarray(w1, np.float32), np.asarray(w2, np.float32),
        np.asarray(w3, np.float32), cfg)

    from concourse.bass_utils import run_bass_kernel_spmd
    res = run_bass_kernel_spmd(nc, in_maps, list(range(NCORES)))
    LAST_RESULTS = res
    return _unpack_output(res.results, counts, cfg, starts)


# revision 4
# speedup vs baseline: 64.0702x; 64.0702x over previous
"""Grouped-experts SwiGLU MoE kernel for Trainium2 (8 NeuronCores).

Problem: T=8192 tokens (pre-sorted into contiguous per-expert blocks of
sizes num_tokens_per_expert), D=1024, H=2816, E=8 experts.
out[t] = (silu(x@w1^T) * (x@w3^T)) @ w2^T  with the owning expert's weights;
tokens past sum(counts) produce zeros.

Sharding: 8-way tensor-parallel split of the hidden dim H (padded
2816 -> 3072 = 24 tiles of 128; each core owns 3 h-tiles of every expert).
Every core processes ALL valid tokens of ALL experts for its h-slice and
emits partial outputs (contraction over h is split); the host sums the 8
partials.  This makes every core's instruction stream identical (true SPMD)
while doing only ceil(count_e/128)-tile work per expert -- perfectly
load-balanced regardless of how unbalanced the expert counts are.

GEMMs run in bf16 (PE 1 cycle/row vs 4 for fp32) with fp32 PSUM accumulation.
"""

import sys

sys.path.insert(0, "/opt/trn_rl_repo")

import numpy as np
import ml_dtypes

T, D, E = 8192, 1024, 8
H = 2816
CAP = T // E
NCORES = 8
HT = 3  # h-tiles of 128 per core (24 total, 22 real + 2 zero pads)
HSLICE = HT * 128  # 384
BF16 = ml_dtypes.bfloat16

_COMPILE_CACHE = {}
LAST_RESULTS = None  # BassKernelResults of the most recent device run


def _derive_cfg(counts):
    """Static structure derived from the per-expert token counts."""
    counts = [int(c) for c in counts]
    tiles = [-(-c // 128) for c in counts]  # ceil(c/128), 0..8
    offs = [0]
    for t in tiles:
        offs.append(offs[-1] + t * 128)
    total_cols = offs[-1]
    # chunks of <=512 packed-x columns, each within a single expert
    chunks = []  # (expert, col0, width)
    for e in range(E):
        cols = tiles[e] * 128
        c0 = 0
        while c0 < cols:
            w = min(512, cols - c0)
            chunks.append((e, offs[e] + c0, w))
            c0 += w
    return {
        "counts": counts,
        "tiles": tiles,
        "offs": offs[:E],
        "total_cols": total_cols,
        "chunks": chunks,
    }


def _build_program(cfg, repeat=1):
    import concourse.bass as bass
    import concourse.bacc as bacc
    import concourse.mybir as mybir
    import concourse.tile as tile

    dt = mybir.dt
    COLS = cfg["total_cols"]
    tiles = cfg["tiles"]
    offs = cfg["offs"]

    nc = bacc.Bacc("TRN2", target_bir_lowering=False, debug=False,
                   num_devices=NCORES)

    xts = nc.dram_tensor("xts", [D, COLS], dt.bfloat16, kind="ExternalInput").ap()
    w1s = nc.dram_tensor("w1s", [E, D, HSLICE], dt.bfloat16, kind="ExternalInput").ap()
    w3s = nc.dram_tensor("w3s", [E, D, HSLICE], dt.bfloat16, kind="ExternalInput").ap()
    w2s = nc.dram_tensor("w2s", [E, HSLICE, D], dt.bfloat16, kind="ExternalInput").ap()
    outp = nc.dram_tensor("outp", [COLS, D], dt.bfloat16, kind="ExternalOutput").ap()

    with tile.TileContext(nc) as tc:
        with (
            tc.tile_pool(name="xpool", bufs=1) as xpool,
            tc.tile_pool(name="wpool", bufs=2) as wpool,
            tc.tile_pool(name="h2pool", bufs=2) as h2pool,
            tc.tile_pool(name="sgpool", bufs=3) as sgpool,
            tc.tile_pool(name="obpool", bufs=3) as obpool,
            tc.tile_pool(name="psgu", bufs=2, space="PSUM") as psgu,
            tc.tile_pool(name="pso", bufs=2, space="PSUM") as pso,
        ):
          for _rep in range(repeat):
            # resident packed-x tiles: d-tile -> (128, COLS)
            xt = []
            for do in range(8):
                t = xpool.tile([128, COLS], dt.bfloat16, tag=f"xt{do}")
                nc.sync.dma_start(t[:], xts[do * 128:(do + 1) * 128, :])
                xt.append(t)

            for e in range(E):
                if tiles[e] == 0:
                    continue
                ecols = tiles[e] * 128
                w1t = wpool.tile([128, 8, HSLICE], dt.bfloat16, tag="w1t")
                nc.sync.dma_start(
                    w1t[:], w1s[e].rearrange("(do p) h -> p do h", p=128))
                w3t = wpool.tile([128, 8, HSLICE], dt.bfloat16, tag="w3t")
                nc.sync.dma_start(
                    w3t[:], w3s[e].rearrange("(do p) h -> p do h", p=128))
                w2t = wpool.tile([128, HT, D], dt.bfloat16, tag="w2t")
                nc.sync.dma_start(
                    w2t[:], w2s[e].rearrange("(ko p) d -> p ko d", p=128))

                h2 = h2pool.tile([128, HT, 1024], dt.bfloat16, tag="h2")
                for h in range(HT):
                    for (ce, col0, w) in cfg["chunks"]:
                        if ce != e:
                            continue
                        rel0 = col0 - offs[e]
                        pg = psgu.tile([128, 512], dt.float32, tag="pg")
                        pu = psgu.tile([128, 512], dt.float32, tag="pu")
                        for d in range(8):
                            nc.tensor.matmul(
                                pg[:, :w],
                                w1t[:, d, h * 128:(h + 1) * 128],
                                xt[d][:, col0:col0 + w],
                                start=(d == 0), stop=(d == 7))
                        for d in range(8):
                            nc.tensor.matmul(
                                pu[:, :w],
                                w3t[:, d, h * 128:(h + 1) * 128],
                                xt[d][:, col0:col0 + w],
                                start=(d == 0), stop=(d == 7))
                        sg = sgpool.tile([128, 512], dt.float32, tag="sg")
                        nc.scalar.activation(
                            sg[:, :w], pg[:, :w],
                            mybir.ActivationFunctionType.Silu)
                        nc.vector.tensor_mul(
                            out=h2[:, h, rel0:rel0 + w],
                            in0=sg[:, :w], in1=pu[:, :w])

                for tt in range(tiles[e]):
                    row0 = offs[e] + tt * 128
                    ob = obpool.tile([128, D], dt.bfloat16, tag="ob")
                    for dh in range(2):
                        po = pso.tile([128, 512], dt.float32, tag="po")
                        for k in range(HT):
                            nc.tensor.matmul(
                                po[:],
                                h2[:, k, tt * 128:(tt + 1) * 128],
                                w2t[:, k, dh * 512:(dh + 1) * 512],
                                start=(k == 0), stop=(k == HT - 1))
                        nc.vector.tensor_copy(
                            out=ob[:, dh * 512:(dh + 1) * 512], in_=po[:])
                    nc.sync.dma_start(outp[row0:row0 + 128, :], ob[:])

    nc.compile()
    return nc


def _get_program(cfg, repeat=1):
    key = (tuple(cfg["tiles"]), repeat)
    if key not in _COMPILE_CACHE:
        _COMPILE_CACHE[key] = _build_program(cfg, repeat)
    return _COMPILE_CACHE[key]


def _pack_inputs(x, counts, w1, w2, w3, cfg):
    """Build per-core input maps (host-side routing + layout)."""
    tiles, offs, COLS = cfg["tiles"], cfg["offs"], cfg["total_cols"]

    # packed x: all valid tokens, per-expert blocks padded to 128 rows
    xpack = np.zeros((COLS, D), np.float32)
    starts = np.concatenate([[0], np.cumsum(counts)]).astype(np.int64)
    for e in range(E):
        c = int(counts[e])
        if c:
            xpack[offs[e]:offs[e] + c] = x[starts[e]:starts[e] + c]
    xts = np.ascontiguousarray(xpack.T.astype(BF16))  # (D, COLS)

    # weights: transpose so the contraction dim leads, pad H to 3072,
    # slice per core
    w1b = w1.astype(BF16)
    w3b = w3.astype(BF16)
    w2b = w2.astype(BF16)
    # (E, D, Hpad)
    w1T = np.zeros((E, D, NCORES * HSLICE), BF16)
    w1T[:, :, :H] = np.transpose(w1b, (0, 2, 1))
    w3T = np.zeros((E, D, NCORES * HSLICE), BF16)
    w3T[:, :, :H] = np.transpose(w3b, (0, 2, 1))
    # (E, Hpad, D)
    w2T = np.zeros((E, NCORES * HSLICE, D), BF16)
    w2T[:, :H, :] = np.transpose(w2b, (0, 2, 1))

    in_maps = []
    for c in range(NCORES):
        hs = slice(c * HSLICE, (c + 1) * HSLICE)
        in_maps.append({
            "xts": xts,
            "w1s": np.ascontiguousarray(w1T[:, :, hs]),
            "w3s": np.ascontiguousarray(w3T[:, :, hs]),
            "w2s": np.ascontiguousarray(w2T[:, hs, :]),
        })
    return in_maps, starts


def _unpack_output(results, counts, cfg, starts):
    offs = cfg["offs"]
    acc = np.zeros((cfg["total_cols"], D), np.float32)
    for r in results:
        acc += r["outp"].astype(np.float32)
    out = np.zeros((T, D), np.float32)
    for e in range(E):
        c = int(counts[e])
        if c:
            out[starts[e]:starts[e] + c] = acc[offs[e]:offs[e] + c]
    return out


def kernel(x, num_tokens_per_expert, w1, w2, w3):
    global LAST_RESULTS
    counts = np.asarray(num_tokens_per_expert).astype(np.int64)
    cfg = _derive_cfg(counts)
    if cfg["total_cols"] == 0:
        return np.zeros((T, D), np.float32)

    nc = _get_program(cfg)
    in_maps, starts = _pack_inputs(
        np.asarray(x, np.float32), counts,
        np.asarray(w1, np.float32), np.asarray(w2, np.float32),
        np.asarray(w3, np.float32), cfg)

    from concourse.bass_utils import run_bass_kernel_spmd
    res = run_bass_kernel_spmd(nc, in_maps, list(range(NCORES)))
    LAST_RESULTS = res
    return _unpack_output(res.results, counts, cfg, starts)


# revision 10
# speedup vs baseline: 90.7907x; 1.4171x over previous
"""Grouped-experts SwiGLU MoE kernel for Trainium2 (8 NeuronCores).

Problem: T=8192 tokens (pre-sorted into contiguous per-expert blocks of
sizes num_tokens_per_expert), D=1024, H=2816, E=8 experts.
out[t] = (silu(x@w1^T) * (x@w3^T)) @ w2^T  with the owning expert's weights;
tokens past sum(counts) produce zeros.

Sharding: 8-way tensor-parallel split of the hidden dim H (padded
2816 -> 3072 = 24 tiles of 128; each core owns 3 h-tiles of every expert).
Every core processes ALL valid tokens of ALL experts for its h-slice and
emits partial outputs (contraction over h is split); the host sums the 8
partials.  This makes every core's instruction stream identical (true SPMD)
while doing only ceil(count_e/128)-tile work per expert -- perfectly
load-balanced regardless of how unbalanced the expert counts are.

GEMMs run in bf16 (PE 1 cycle/row vs 4 for fp32) with fp32 PSUM accumulation.
"""

import sys

sys.path.insert(0, "/opt/trn_rl_repo")

import numpy as np
import ml_dtypes

T, D, E = 8192, 1024, 8
H = 2816
CAP = T // E
NCORES = 8
HT = 3  # h-tiles of 128 per core (24 total, 22 real + 2 zero pads)
HSLICE = HT * 128  # 384
BF16 = ml_dtypes.bfloat16

_COMPILE_CACHE = {}
LAST_RESULTS = None  # BassKernelResults of the most recent device run


def _derive_cfg(counts):
    """Static structure derived from the per-expert token counts.
    Tokens are packed exactly (no padding): expert e owns packed columns
    [offs[e], offs[e]+counts[e]).  GEMM1/3 consume near-equal chunks of
    <=512 columns; GEMM2 emits tiles of <=128 token rows."""
    counts = [int(c) for c in counts]
    offs = [0]
    for c in counts:
        offs.append(offs[-1] + c)
    total_cols = offs[-1]
    chunks = []   # (expert, col0, width<=512) for GEMM1/3
    ttiles = []   # (expert, col0, m<=128) for GEMM2 output tiles
    for e in range(E):
        c = counts[e]
        if c == 0:
            continue
        n = -(-c // 512)
        base, rem = divmod(c, n)
        c0 = 0
        for i in range(n):
            w = base + (1 if i < rem else 0)
            chunks.append((e, offs[e] + c0, w))
            c0 += w
        c0 = 0
        while c0 < c:
            m = min(128, c - c0)
            ttiles.append((e, offs[e] + c0, m))
            c0 += m
    return {
        "counts": counts,
        "offs": offs[:E],
        "total_cols": total_cols,
        "chunks": chunks,
        "ttiles": ttiles,
    }


def _build_program(cfg, repeat=1):
    import concourse.bass as bass
    import concourse.bacc as bacc
    import concourse.mybir as mybir
    import concourse.tile as tile

    dt = mybir.dt
    COLS = cfg["total_cols"]
    counts = cfg["counts"]
    offs = cfg["offs"]

    nc = bacc.Bacc("TRN2", target_bir_lowering=False, debug=False,
                   num_devices=NCORES)

    xts = nc.dram_tensor("xts", [D, COLS], dt.bfloat16, kind="ExternalInput").ap()
    w1s = nc.dram_tensor("w1s", [E, D, HSLICE], dt.bfloat16, kind="ExternalInput").ap()
    w3s = nc.dram_tensor("w3s", [E, D, HSLICE], dt.bfloat16, kind="ExternalInput").ap()
    w2s = nc.dram_tensor("w2s", [E, HSLICE, D], dt.bfloat16, kind="ExternalInput").ap()
    outp = nc.dram_tensor("outp", [COLS, D], dt.bfloat16, kind="ExternalOutput").ap()

    with tile.TileContext(nc) as tc:
        with (
            tc.tile_pool(name="xpool", bufs=1) as xpool,
            tc.tile_pool(name="wpool", bufs=2) as wpool,
            tc.tile_pool(name="h2pool", bufs=2) as h2pool,
            tc.tile_pool(name="sgpool", bufs=3) as sgpool,
            tc.tile_pool(name="obpool", bufs=3) as obpool,
            tc.tile_pool(name="psgu", bufs=2, space="PSUM") as psgu,
            tc.tile_pool(name="pso", bufs=2, space="PSUM") as pso,
        ):
          for _rep in range(repeat):
            # resident packed-x tiles: d-tile -> (128, COLS)
            xt = []
            for do in range(8):
                t = xpool.tile([128, COLS], dt.bfloat16, tag=f"xt{do}")
                nc.sync.dma_start(t[:], xts[do * 128:(do + 1) * 128, :])
                xt.append(t)

            for e in range(E):
                if counts[e] == 0:
                    continue
                w1t = wpool.tile([128, 8, HSLICE], dt.bfloat16, tag="w1t")
                nc.sync.dma_start(
                    w1t[:], w1s[e].rearrange("(do p) h -> p do h", p=128))
                w3t = wpool.tile([128, 8, HSLICE], dt.bfloat16, tag="w3t")
                nc.sync.dma_start(
                    w3t[:], w3s[e].rearrange("(do p) h -> p do h", p=128))
                w2t = wpool.tile([128, HT, D], dt.bfloat16, tag="w2t")
                nc.sync.dma_start(
                    w2t[:], w2s[e].rearrange("(ko p) d -> p ko d", p=128))

                h2 = h2pool.tile([128, HT, 1024], dt.bfloat16, tag="h2")
                for h in range(HT):
                    for (ce, col0, w) in cfg["chunks"]:
                        if ce != e:
                            continue
                        rel0 = col0 - offs[e]
                        pg = psgu.tile([128, 512], dt.float32, tag="pg")
                        pu = psgu.tile([128, 512], dt.float32, tag="pu")
                        for d in range(8):
                            nc.tensor.matmul(
                                pg[:, :w],
                                w1t[:, d, h * 128:(h + 1) * 128],
                                xt[d][:, col0:col0 + w],
                                start=(d == 0), stop=(d == 7))
                        for d in range(8):
                            nc.tensor.matmul(
                                pu[:, :w],
                                w3t[:, d, h * 128:(h + 1) * 128],
                                xt[d][:, col0:col0 + w],
                                start=(d == 0), stop=(d == 7))
                        sg = sgpool.tile([128, 512], dt.float32, tag="sg")
                        nc.scalar.activation(
                            sg[:, :w], pg[:, :w],
                            mybir.ActivationFunctionType.Silu)
                        nc.vector.tensor_mul(
                            out=h2[:, h, rel0:rel0 + w],
                            in0=sg[:, :w], in1=pu[:, :w])

                for (te, col0, m) in cfg["ttiles"]:
                    if te != e:
                        continue
                    rel0 = col0 - offs[e]
                    ob = obpool.tile([128, D], dt.bfloat16, tag="ob")
                    for dh in range(2):
                        po = pso.tile([128, 512], dt.float32, tag="po")
                        for k in range(HT):
                            nc.tensor.matmul(
                                po[:m],
                                h2[:, k, rel0:rel0 + m],
                                w2t[:, k, dh * 512:(dh + 1) * 512],
                                start=(k == 0), stop=(k == HT - 1))
                        nc.vector.tensor_copy(
                            out=ob[:m, dh * 512:(dh + 1) * 512], in_=po[:m])
                    nc.sync.dma_start(outp[col0:col0 + m, :], ob[:m])

    nc.compile()
    return nc


def _get_program(cfg, repeat=1):
    key = (tuple(cfg["counts"]), repeat)
    if key not in _COMPILE_CACHE:
        _COMPILE_CACHE[key] = _build_program(cfg, repeat)
    return _COMPILE_CACHE[key]


def _pack_inputs(x, counts, w1, w2, w3, cfg):
    """Build per-core input maps (host-side routing + layout)."""
    offs, COLS = cfg["offs"], cfg["total_cols"]

    # packed x: all valid tokens, exactly packed per expert
    xpack = np.zeros((COLS, D), np.float32)
    starts = np.concatenate([[0], np.cumsum(counts)]).astype(np.int64)
    for e in range(E):
        c = int(counts[e])
        if c:
            xpack[offs[e]:offs[e] + c] = x[starts[e]:starts[e] + c]
    xts = np.ascontiguousarray(xpack.T.astype(BF16))  # (D, COLS)

    # weights: transpose so the contraction dim leads, pad H to 3072,
    # slice per core
    w1b = w1.astype(BF16)
    w3b = w3.astype(BF16)
    w2b = w2.astype(BF16)
    # (E, D, Hpad)
    w1T = np.zeros((E, D, NCORES * HSLICE), BF16)
    w1T[:, :, :H] = np.transpose(w1b, (0, 2, 1))
    w3T = np.zeros((E, D, NCORES * HSLICE), BF16)
    w3T[:, :, :H] = np.transpose(w3b, (0, 2, 1))
    # (E, Hpad, D)
    w2T = np.zeros((E, NCORES * HSLICE, D), BF16)
    w2T[:, :H, :] = np.transpose(w2b, (0, 2, 1))

    in_maps = []
    for c in range(NCORES):
        hs = slice(c * HSLICE, (c + 1) * HSLICE)
        in_maps.append({
            "xts": xts,
            "w1s": np.ascontiguousarray(w1T[:, :, hs]),
            "w3s": np.ascontiguousarray(w3T[:, :, hs]),
            "w2s": np.ascontiguousarray(w2T[:, hs, :]),
        })
    return in_maps, starts


def _unpack_output(results, counts, cfg, starts):
    offs = cfg["offs"]
    acc = np.zeros((cfg["total_cols"], D), np.float32)
    for r in results:
        acc += r["outp"].astype(np.float32)
    out = np.zeros((T, D), np.float32)
    for e in range(E):
        c = int(counts[e])
        if c:
            out[starts[e]:starts[e] + c] = acc[offs[e]:offs[e] + c]
    return out


def kernel(x, num_tokens_per_expert, w1, w2, w3):
    global LAST_RESULTS
    counts = np.asarray(num_tokens_per_expert).astype(np.int64)
    cfg = _derive_cfg(counts)
    if cfg["total_cols"] == 0:
        return np.zeros((T, D), np.float32)

    nc = _get_program(cfg)
    in_maps, starts = _pack_inputs(
        np.asarray(x, np.float32), counts,
        np.asarray(w1, np.float32), np.asarray(w2, np.float32),
        np.asarray(w3, np.float32), cfg)

    from concourse.bass_utils import run_bass_kernel_spmd
    res = run_bass_kernel_spmd(nc, in_maps, list(range(NCORES)))
    LAST_RESULTS = res
    return _unpack_output(res.results, counts, cfg, starts)
